# revision 10
# baseline (speedup 1.0000x reference)
"""Trainium2 Bass kernel for AtomGCNLayer (ResGatedGraphConv + BatchNorm + ReLU + residual).

v4: single fused kernel (8 NeuronCores, SPMD), scatter grouped by PE tile.
  - Host: degree-balanced node->window packing (B=5, ~1.6% slot padding):
    nodes sorted by in-degree, dealt round-robin into 8192 windows of <=64
    nodes so every window has nearly equal edge count.
  - Device main loop: per 128-edge block one bf16 matmul computes gate-arg
    and v ([128,32] PSUM).  ACT: sigmoid + v-copy to SBUF bf16; gpsimd:
    msg = sg*v; DVE: one-hot scatter matrix (2 groups per instruction);
    PE: OH^T scatter matmuls + fused skip matmul per 64-node window; BN
    partial sums via ones-matmuls.  Per-bank h stays resident in SBUF.
  - Tail: BN stats cross-core reduction via in-kernel HBM AllReduce
    (gpsimd), scale/shift computed on-device and broadcast via PE, then
    BN+ReLU+residual applied from SBUF-resident h.  One kernel launch;
    h never round-trips to HBM.
"""

import math

import numpy as np
import ml_dtypes

from concourse import bacc, mybir
import concourse.tile as tile
from concourse.bass_utils import run_bass_kernel_spmd

BF16 = ml_dtypes.bfloat16

N = 500000
E = 5000000
D = 16
NC = 8
W = 64            # nodes per scatter window
PW = 1024         # windows per core
NODES_C = W * PW  # 65536 nodes per core
NPAD = NC * NODES_C
NWIN = NC * PW
NBANK = PW // 64  # 16 agg banks per core (64 windows per bank)
BN_EPS = 1e-5

_nc_cache = {}
_gslot = None     # node -> global padded slot, set by host_prep


def _build_phase1(B):
    GB = 2 * B            # blocks per group (2 windows per group)
    SC = 8                # groups per DMA superchunk
    gpb = 32              # groups per agg bank
    nsc_pb = gpb // SC    # superchunks per bank
    nsc = NBANK * nsc_pb
    S_c = PW * B * 128

    bf = mybir.dt.bfloat16
    f32 = mybir.dt.float32
    nc = bacc.Bacc(None, target_bir_lowering=False, debug=True, num_devices=NC)
    INP = nc.dram_tensor("inp", [49, S_c // 128, 128], bf, kind="ExternalInput")
    TREL = nc.dram_tensor("trel", [nsc, 128, SC, GB], bf, kind="ExternalInput")
    IOTA = nc.dram_tensor("iota", [128, W, 2, GB], bf, kind="ExternalInput")
    WGT = nc.dram_tensor("wgt", [49, 32], bf, kind="ExternalInput")
    XSK = nc.dram_tensor("xsk", [17, NODES_C], f32, kind="ExternalInput")
    WSB = nc.dram_tensor("wsb", [17, 48], f32, kind="ExternalInput")
    XT = nc.dram_tensor("xt", [NBANK, 128, 32, 16], bf, kind="ExternalInput")
    Y = nc.dram_tensor("y", [NBANK, 128, 32, 16], f32, kind="ExternalOutput")
    STATS = nc.dram_tensor("stats", [1, 1024], f32, kind="ExternalOutput")
    CCIN = nc.dram_tensor("ccin", [1, 1024], f32)
    CCOUT = nc.dram_tensor("ccout", [1, 1024], f32, addr_space="Shared")

    SIG = mybir.ActivationFunctionType.Sigmoid
    SQ = mybir.ActivationFunctionType.Square
    CP = mybir.ActivationFunctionType.Copy
    SQRT = mybir.ActivationFunctionType.Sqrt
    RELU = mybir.ActivationFunctionType.Relu
    MUL = mybir.AluOpType.mult
    ADD = mybir.AluOpType.add
    SUB = mybir.AluOpType.subtract
    EQ = mybir.AluOpType.is_equal

    with tile.TileContext(nc) as tc:
        with (
            tc.tile_pool(name="const", bufs=1) as cpool,
            tc.tile_pool(name="sbuf", bufs=3) as pool,
            tc.tile_pool(name="xskp", bufs=2) as xpool,
            tc.tile_pool(name="hres", bufs=NBANK) as hpool,
            tc.tile_pool(name="xres", bufs=NBANK) as x2pool,
            tc.tile_pool(name="tail", bufs=2) as tpool,
            tc.tile_pool(name="pm", bufs=2, space="PSUM") as pm,
            tc.tile_pool(name="pa", bufs=2, space="PSUM") as pa,
            tc.tile_pool(name="psk", bufs=1, space="PSUM") as psk,
            tc.tile_pool(name="pst", bufs=1, space="PSUM") as pst,
        ):
            wt = cpool.tile([49, 32], bf)
            nc.sync.dma_start(wt[:], WGT[:])
            wsb = cpool.tile([17, 48], f32)
            nc.sync.dma_start(wsb[:], WSB[:])
            it = cpool.tile([128, W, 2, GB], bf)
            nc.sync.dma_start(it[:], IOTA[:])
            ones = cpool.tile([128, 1], f32)
            nc.gpsimd.memset(ones[:], 1.0)
            onesb = cpool.tile([128, 1], bf)
            nc.gpsimd.memset(onesb[:], 1.0)
            ones1 = cpool.tile([1, 128], f32)
            nc.gpsimd.memset(ones1[:], 1.0)

            sstat = pst.tile([33, 512], f32, space="PSUM", tag="sstat")

            hsbs = []
            xbs = []
            for k in range(NBANK):
                agg = pa.tile([128, 32, 16], f32, space="PSUM", tag="agg")
                xsk = xpool.tile([17, 64, W], f32, tag="xsk")
                nc.sync.dma_start(xsk[:], XSK[:, k * 4096:(k + 1) * 4096])
                xb = x2pool.tile([128, 32, 16], bf, tag="xb")
                nc.sync.dma_start(xb[:], XT[k])
                xbs.append(xb)
                for ss in range(nsc_pb):
                    sc = k * nsc_pb + ss
                    ic = pool.tile([49, SC * GB, 128], bf, tag="ic", name="ic",
                                   bufs=2)
                    ic_eng = nc.sync if ss % 2 == 0 else nc.gpsimd
                    ic_eng.dma_start(ic[:], INP[:, sc * SC * GB:(sc + 1) * SC * GB, :])
                    tct8 = pool.tile([128, SC, GB], bf, tag="tct")
                    nc.sync.dma_start(tct8[:], TREL[sc])
                    ohs, msgs = [], []
                    for pp in range(SC // 2):
                        mm = pm.tile([128, 2, GB, 32], f32, space="PSUM",
                                     tag="mm", name="mm")
                        for i in range(2):
                            for b in range(GB):
                                nc.tensor.matmul(mm[:, i, b, :],
                                                 lhsT=ic[:, (2 * pp + i) * GB + b, :],
                                                 rhs=wt[:], start=True, stop=True)
                        sg = pool.tile([128, 2, GB, 16], bf, tag="sg", name="sg",
                                       bufs=6)
                        nc.scalar.activation(sg[:], mm[:, :, :, 0:16], func=SIG)
                        vb = pool.tile([128, 2, GB, 16], bf, tag="vb", name="vb",
                                       bufs=6)
                        nc.scalar.activation(vb[:], mm[:, :, :, 16:32], func=CP)
                        oh = pool.tile([128, W, 2, GB], bf, tag="oh", name="oh",
                                       bufs=6)
                        nc.vector.tensor_tensor(
                            oh[:],
                            tct8[:, 2 * pp:2 * pp + 2, :].unsqueeze(1)
                                .to_broadcast([128, W, 2, GB]),
                            it[:],
                            op=EQ,
                        )
                        msg = pool.tile([128, 2, GB, 16], bf, tag="msg",
                                        name="msg", bufs=6)
                        nc.gpsimd.tensor_tensor(msg[:], sg[:], vb[:], op=MUL)
                        ohs.append(oh)
                        msgs.append(msg)
                    # scatter matmuls: whole-window accumulation groups,
                    # windows ordered by PE tile position (2 config switches
                    # per superchunk); skip term goes to its own PSUM bank
                    for wi in range(2):
                        pos = 64 * wi
                        for pp in range(SC // 2):
                            for i in range(2):
                                gg = ss * SC + 2 * pp + i
                                win_b = gg * 2 + wi
                                col = win_b // 2
                                out_ap = agg[pos:pos + 64, col, :]
                                for b in range(B):
                                    blk = wi * B + b
                                    nc.tensor.matmul(out_ap,
                                                     lhsT=ohs[pp][:, :, i, blk],
                                                     rhs=msgs[pp][:, i, blk, :],
                                                     start=(b == 0),
                                                     stop=(b == B - 1),
                                                     tile_position=(0, pos))
                # skip matmuls: single-matmul groups into a separate bank,
                # uniform tile config, grouped by position
                skp = psk.tile([128, 32, 16], f32, space="PSUM", tag="skp",
                               name="skp")
                for wi in range(2):
                    pos = 64 * wi
                    for cc in range(32):
                        win_b = 2 * cc + wi
                        nc.tensor.matmul(skp[pos:pos + 64, cc, :],
                                         lhsT=xsk[:, win_b, :],
                                         rhs=wsb[:, 0:16],
                                         start=True, stop=True,
                                         tile_position=(0, pos))
                sksb = pool.tile([128, 32, 16], f32, tag="sksb", name="sksb")
                nc.scalar.activation(sksb[:], skp[:], func=CP)
                hsb = hpool.tile([128, 32, 16], f32, tag="hsb")
                nc.vector.tensor_tensor(hsb[:], agg[:], sksb[:], op=ADD)
                hsbs.append(hsb)
                hsq = pool.tile([128, 32, 16], bf, tag="hsq")
                nc.scalar.activation(hsq[:], hsb[:], func=SQ)
                nc.tensor.matmul(sstat[0:1, :], lhsT=ones[:], rhs=hsb[:],
                                 start=(k == 0), stop=(k == NBANK - 1),
                                 tile_position=(0, 0), skip_group_check=True)
                nc.tensor.matmul(sstat[32:33, :], lhsT=onesb[:], rhs=hsq[:],
                                 start=(k == 0), stop=(k == NBANK - 1),
                                 tile_position=(0, 32), skip_group_check=True)

            # ---- stats AllReduce across the 8 cores ----
            stsb0 = pool.tile([1, 512], f32, tag="stsb0")
            nc.vector.tensor_copy(stsb0[:], sstat[0:1, :])
            nc.sync.dma_start(CCIN[0:1, 0:512], stsb0[:])
            stsb1 = pool.tile([1, 512], f32, tag="stsb1")
            nc.vector.tensor_copy(stsb1[:], sstat[32:33, :])
            nc.sync.dma_start(CCIN[0:1, 512:1024], stsb1[:])
            nc.gpsimd.collective_compute(
                "AllReduce", ADD, replica_groups=[list(range(NC))],
                ins=[CCIN[:]], outs=[CCOUT[:]],
            )
            # sred layout: [b(2) x col(32) x f(16)], b=0 sum(h), b=1 sum(h^2)
            sred = pool.tile([1, 2, 32, 16], f32, tag="sred")
            nc.sync.dma_start(sred[:], CCOUT[:])
            nc.sync.dma_start(STATS[:], sred[:])

            # ---- on-device scale/shift: binary-tree col reduction ----
            tr1 = pool.tile([1, 2, 16, 16], f32, tag="tr1")
            nc.vector.tensor_tensor(tr1[:], sred[:, :, 0:16, :],
                                    sred[:, :, 16:32, :], op=ADD)
            tr2 = pool.tile([1, 2, 8, 16], f32, tag="tr2")
            nc.vector.tensor_tensor(tr2[:], tr1[:, :, 0:8, :],
                                    tr1[:, :, 8:16, :], op=ADD)
            tr3 = pool.tile([1, 2, 4, 16], f32, tag="tr3")
            nc.vector.tensor_tensor(tr3[:], tr2[:, :, 0:4, :],
                                    tr2[:, :, 4:8, :], op=ADD)
            tr4 = pool.tile([1, 2, 2, 16], f32, tag="tr4")
            nc.vector.tensor_tensor(tr4[:], tr3[:, :, 0:2, :],
                                    tr3[:, :, 2:4, :], op=ADD)
            ms = pool.tile([1, 2, 16], f32, tag="ms")   # [mean | E[h^2]]
            nc.vector.tensor_tensor(ms[:].unsqueeze(2),
                                    tr4[:, :, 0:1, :], tr4[:, :, 1:2, :], op=ADD)
            nc.vector.tensor_scalar(ms[:], ms[:], 1.0 / N, None, op0=MUL)
            m2 = pool.tile([1, 16], f32, tag="m2")
            nc.vector.tensor_tensor(m2[:], ms[:, 0, :], ms[:, 0, :], op=MUL)
            ve = pool.tile([1, 16], f32, tag="ve")
            nc.vector.tensor_tensor(ve[:], ms[:, 1, :], m2[:], op=SUB)
            nc.vector.tensor_scalar(ve[:], ve[:], BN_EPS, None, op0=ADD)
            inv = pool.tile([1, 16], f32, tag="inv")
            nc.vector.reciprocal(inv[:], ve[:])
            rs = pool.tile([1, 16], f32, tag="rs")
            nc.scalar.activation(rs[:], inv[:], func=SQRT)
            ss2 = pool.tile([1, 32], f32, tag="ss2")    # [scale | shift]
            nc.vector.tensor_tensor(ss2[:, 0:16], rs[:], wsb[0:1, 16:32], op=MUL)
            msc = pool.tile([1, 16], f32, tag="msc")
            nc.vector.tensor_tensor(msc[:], ms[:, 0, :], ss2[:, 0:16], op=MUL)
            nc.vector.tensor_tensor(ss2[:, 16:32], wsb[0:1, 32:48], msc[:], op=SUB)
            bcp = pm.tile([128, 32], f32, space="PSUM", tag="mm", name="bcp")
            nc.tensor.matmul(bcp[:], lhsT=ones1[:], rhs=ss2[:],
                             start=True, stop=True, tile_position=(0, 0),
                             skip_group_check=True)
            bcs = cpool.tile([128, 32], f32)
            nc.scalar.activation(bcs[:], bcp[:], func=CP)
            scl_b = bcs[:, 0:16].unsqueeze(1).to_broadcast([128, 32, 16])
            sft_b = bcs[:, 16:32].unsqueeze(1).to_broadcast([128, 32, 16])

            # ---- BN apply + ReLU + residual from SBUF-resident h ----
            for k in range(NBANK):
                hsb = hsbs[k]
                nc.vector.tensor_tensor(hsb[:], hsb[:], scl_b, op=MUL)
                nc.vector.tensor_tensor(hsb[:], hsb[:], sft_b, op=ADD)
                nc.scalar.activation(hsb[:], hsb[:], func=RELU)
                yb = tpool.tile([128, 32, 16], f32, tag="yb")
                nc.gpsimd.tensor_tensor(yb[:], hsb[:], xbs[k][:], op=ADD)
                nc.sync.dma_start(Y[k], yb[:])
    nc.compile()
    return nc


def host_prep(x, edge_index, edge_attr):
    """Degree-balanced window packing + per-core device array layout."""
    global _gslot
    src = np.asarray(edge_index[0], dtype=np.int64)
    tgt = np.asarray(edge_index[1], dtype=np.int64)
    x = np.asarray(x, dtype=np.float32)
    ea = np.asarray(edge_attr, dtype=np.float32)

    # --- node -> (window, pos) by round-robin deal of degree-sorted nodes ---
    deg = np.bincount(tgt, minlength=N)
    order = np.argsort(-deg, kind="stable")
    node2win = np.empty(N, np.int64)
    node2pos = np.empty(N, np.int64)
    node2win[order] = np.arange(N, dtype=np.int64) % NWIN
    node2pos[order] = np.arange(N, dtype=np.int64) // NWIN
    _gslot = node2win * W + node2pos

    wcnt = np.zeros(NWIN, np.int64)
    np.add.at(wcnt, node2win, deg)
    B = max(1, int(math.ceil(wcnt.max() / 128)))
    S_w = 128 * B
    S = NWIN * S_w
    S_c = PW * S_w

    # --- edge slots: sort edges by target window ---
    ew = node2win[tgt]
    perm = np.argsort(ew, kind="stable")
    ew_s = ew[perm]
    starts = np.zeros(NWIN + 1, np.int64)
    starts[1:] = np.cumsum(wcnt)
    slots = ew_s * S_w + (np.arange(E, dtype=np.int64) - starts[ew_s])

    GB = 2 * B
    SC = 8
    nsc_pb = 32 // SC
    nsc = NBANK * nsc_pb

    tgt_s = tgt[perm]
    src_s = src[perm]
    x16 = x.astype(BF16)
    pay = np.zeros((S, 48), BF16)
    pay[slots, 0:16] = x16[tgt_s]
    pay[slots, 16:32] = x16[src_s]
    pay[slots, 32:48] = ea[perm].astype(BF16)

    trel = np.full(S, -1.0, np.float32)
    trel[slots] = node2pos[tgt_s].astype(np.float32)
    trel16 = trel.astype(BF16)

    # x in padded-slot order
    xpad = np.zeros((NPAD, D), np.float32)
    xpad[_gslot] = x
    mask = np.zeros(NPAD, np.float32)
    mask[_gslot] = 1.0

    iota = np.broadcast_to(
        np.arange(W, dtype=np.float32).astype(BF16).reshape(1, W, 1, 1),
        (128, W, 2, GB)).copy()

    xt = x_tiled(xpad)

    in_maps = []
    for c in range(NC):
        inp_c = np.empty((49, S_c), BF16)
        inp_c[0:48] = pay[c * S_c:(c + 1) * S_c].T
        inp_c[48] = BF16(1.0)
        inp_c = inp_c.reshape(49, S_c // 128, 128)
        trel_c = (trel16[c * S_c:(c + 1) * S_c]
                  .reshape(nsc, SC, GB, 128).transpose(0, 3, 1, 2).copy())
        xsk_c = np.empty((17, NODES_C), np.float32)
        xsk_c[0:16] = xpad[c * NODES_C:(c + 1) * NODES_C].T
        xsk_c[16] = mask[c * NODES_C:(c + 1) * NODES_C]
        in_maps.append({
            "inp": inp_c, "trel": trel_c, "iota": iota,
            "xsk": xsk_c, "xt": xt[c],
        })
    return B, in_maps, xpad


def weight_arrays(Wk, bk, Wq, bq, Wv, bv, We, Ws, bs, bias,
                  gamma=None, beta=None):
    wgt = np.zeros((49, 32), np.float32)
    wgt[0:16, 0:16] = Wk
    wgt[16:32, 0:16] = Wq
    wgt[32:48, 0:16] = We
    wgt[48, 0:16] = bk + bq
    wgt[16:32, 16:32] = Wv
    wgt[48, 16:32] = bv
    wsb = np.zeros((17, 48), np.float32)
    wsb[0:16, 0:16] = Ws
    wsb[16, 0:16] = bs + bias
    wsb[0, 16:32] = 1.0 if gamma is None else np.asarray(gamma, np.float32)
    wsb[0, 32:48] = 0.0 if beta is None else np.asarray(beta, np.float32)
    return wgt.astype(BF16), wsb


def x_tiled(xpad):
    # [NC, NBANK, 128, 32, 16]; slot s in core c: s = k*4096 + col*128 + p
    xt = xpad.reshape(NC, NBANK, 32, 128, D).transpose(0, 1, 3, 2, 4)
    return np.ascontiguousarray(xt).astype(BF16)


def untile_y(y_t):
    # y_t: [NC, NBANK, 128, 32, 16] -> [NPAD, 16]
    return y_t.transpose(0, 1, 3, 2, 4).reshape(NPAD, D)


def kernel(**inputs):
    x = np.asarray(inputs["x"], np.float32)
    B, in_maps, xpad = host_prep(x, inputs["edge_index"], inputs["edge_attr"])
    wgt, wsb = weight_arrays(
        np.asarray(inputs["Wk"], np.float32), np.asarray(inputs["bk"], np.float32),
        np.asarray(inputs["Wq"], np.float32), np.asarray(inputs["bq"], np.float32),
        np.asarray(inputs["Wv"], np.float32), np.asarray(inputs["bv"], np.float32),
        np.asarray(inputs["We"], np.float32), np.asarray(inputs["Ws"], np.float32),
        np.asarray(inputs["bs"], np.float32), np.asarray(inputs["bias"], np.float32),
        np.asarray(inputs["gamma"], np.float32), np.asarray(inputs["beta"], np.float32))
    for m in in_maps:
        m["wgt"] = wgt
        m["wsb"] = wsb

    if ("p1", B) not in _nc_cache:
        _nc_cache[("p1", B)] = _build_phase1(B)
    nc1 = _nc_cache[("p1", B)]
    res1 = run_bass_kernel_spmd(nc1, in_maps, list(range(NC)))

    y_t = np.stack([res1.results[c]["y"] for c in range(NC)])
    y = untile_y(y_t)[_gslot]
    return y.astype(np.float32)


# revision 11
# speedup vs baseline: 1.3268x; 1.3268x over previous
"""Trainium2 Bass kernel for AtomGCNLayer (ResGatedGraphConv + BatchNorm + ReLU + residual).

v4: single fused kernel (8 NeuronCores, SPMD), scatter grouped by PE tile.
  - Host: degree-balanced node->window packing (B=5, ~1.6% slot padding):
    nodes sorted by in-degree, dealt round-robin into 8192 windows of <=64
    nodes so every window has nearly equal edge count.
  - Device main loop: per 128-edge block one bf16 matmul computes gate-arg
    and v ([128,32] PSUM).  ACT: sigmoid + v-copy to SBUF bf16; gpsimd:
    msg = sg*v; DVE: one-hot scatter matrix (2 groups per instruction);
    PE: OH^T scatter matmuls + fused skip matmul per 64-node window; BN
    partial sums via ones-matmuls.  Per-bank h stays resident in SBUF.
  - Tail: BN stats cross-core reduction via in-kernel HBM AllReduce
    (gpsimd), scale/shift computed on-device and broadcast via PE, then
    BN+ReLU+residual applied from SBUF-resident h.  One kernel launch;
    h never round-trips to HBM.
"""

import math

import numpy as np
import ml_dtypes

from concourse import bacc, mybir
import concourse.tile as tile
from concourse.bass_utils import run_bass_kernel_spmd

BF16 = ml_dtypes.bfloat16

N = 500000
E = 5000000
D = 16
NC = 8
W = 64            # nodes per scatter window
PW = 1024         # windows per core
NODES_C = W * PW  # 65536 nodes per core
NPAD = NC * NODES_C
NWIN = NC * PW
NBANK = PW // 64  # 16 agg banks per core (64 windows per bank)
BN_EPS = 1e-5

_nc_cache = {}
_gslot = None     # node -> global padded slot, set by host_prep


def _build_phase1(B):
    GB = 2 * B            # blocks per group (2 windows per group)
    SC = 8                # groups per DMA superchunk
    gpb = 32              # groups per agg bank
    nsc_pb = gpb // SC    # superchunks per bank
    nsc = NBANK * nsc_pb
    S_c = PW * B * 128

    bf = mybir.dt.bfloat16
    f32 = mybir.dt.float32
    nc = bacc.Bacc(None, target_bir_lowering=False, debug=True, num_devices=NC)
    INP = nc.dram_tensor("inp", [49, S_c // 128, 128], bf, kind="ExternalInput")
    TREL = nc.dram_tensor("trel", [nsc, 128, SC, GB], bf, kind="ExternalInput")
    IOTA = nc.dram_tensor("iota", [128, W, 2, GB], bf, kind="ExternalInput")
    WGT = nc.dram_tensor("wgt", [49, 32], bf, kind="ExternalInput")
    XSK = nc.dram_tensor("xsk", [17, NODES_C], f32, kind="ExternalInput")
    WSB = nc.dram_tensor("wsb", [17, 48], f32, kind="ExternalInput")
    XT = nc.dram_tensor("xt", [NBANK, 128, 32, 16], bf, kind="ExternalInput")
    Y = nc.dram_tensor("y", [NBANK, 128, 32, 16], f32, kind="ExternalOutput")
    STATS = nc.dram_tensor("stats", [1, 1024], f32, kind="ExternalOutput")
    CCIN = nc.dram_tensor("ccin", [1, 1024], f32)
    CCOUT = nc.dram_tensor("ccout", [1, 1024], f32, addr_space="Shared")

    SIG = mybir.ActivationFunctionType.Sigmoid
    SQ = mybir.ActivationFunctionType.Square
    CP = mybir.ActivationFunctionType.Copy
    SQRT = mybir.ActivationFunctionType.Sqrt
    RELU = mybir.ActivationFunctionType.Relu
    MUL = mybir.AluOpType.mult
    ADD = mybir.AluOpType.add
    SUB = mybir.AluOpType.subtract
    EQ = mybir.AluOpType.is_equal

    with tile.TileContext(nc) as tc:
        with (
            tc.tile_pool(name="const", bufs=1) as cpool,
            tc.tile_pool(name="sbuf", bufs=3) as pool,
            tc.tile_pool(name="xskp", bufs=2) as xpool,
            tc.tile_pool(name="hres", bufs=NBANK) as hpool,
            tc.tile_pool(name="xres", bufs=NBANK) as x2pool,
            tc.tile_pool(name="tail", bufs=2) as tpool,
            tc.tile_pool(name="pm", bufs=2, space="PSUM") as pm,
            tc.tile_pool(name="pa", bufs=2, space="PSUM") as pa,
            tc.tile_pool(name="pst", bufs=1, space="PSUM") as pst,
        ):
            wt = cpool.tile([49, 32], bf)
            nc.sync.dma_start(wt[:], WGT[:])
            wsb = cpool.tile([17, 48], f32)
            nc.sync.dma_start(wsb[:], WSB[:])
            it = cpool.tile([128, W, 2, GB], bf)
            nc.sync.dma_start(it[:], IOTA[:])
            ones = cpool.tile([128, 1], f32)
            nc.gpsimd.memset(ones[:], 1.0)
            onesb = cpool.tile([128, 1], bf)
            nc.gpsimd.memset(onesb[:], 1.0)
            ones1 = cpool.tile([1, 128], f32)
            nc.gpsimd.memset(ones1[:], 1.0)

            sstat = pst.tile([33, 512], f32, space="PSUM", tag="sstat")

            hsbs = []
            xbs = []
            for k in range(NBANK):
                agg = pa.tile([128, 32, 16], f32, space="PSUM", tag="agg")
                xsk = xpool.tile([17, 64, W], f32, tag="xsk")
                nc.sync.dma_start(xsk[:], XSK[:, k * 4096:(k + 1) * 4096])
                xb = x2pool.tile([128, 32, 16], bf, tag="xb")
                nc.sync.dma_start(xb[:], XT[k])
                xbs.append(xb)
                for ss in range(nsc_pb):
                    sc = k * nsc_pb + ss
                    ic = pool.tile([49, SC * GB, 128], bf, tag="ic", name="ic",
                                   bufs=2)
                    ic_eng = nc.sync if ss % 2 == 0 else nc.gpsimd
                    ic_eng.dma_start(ic[:], INP[:, sc * SC * GB:(sc + 1) * SC * GB, :])
                    tct8 = pool.tile([128, SC, GB], bf, tag="tct")
                    nc.sync.dma_start(tct8[:], TREL[sc])
                    ohs, msgs = [], []
                    for pp in range(SC // 2):
                        mm = pm.tile([128, 2, GB, 32], f32, space="PSUM",
                                     tag="mm", name="mm")
                        for i in range(2):
                            for b in range(GB):
                                nc.tensor.matmul(mm[:, i, b, :],
                                                 lhsT=ic[:, (2 * pp + i) * GB + b, :],
                                                 rhs=wt[:], start=True, stop=True)
                        sg = pool.tile([128, 2, GB, 16], bf, tag="sg", name="sg",
                                       bufs=6)
                        nc.scalar.activation(sg[:], mm[:, :, :, 0:16], func=SIG)
                        vb = pool.tile([128, 2, GB, 16], bf, tag="vb", name="vb",
                                       bufs=6)
                        nc.scalar.activation(vb[:], mm[:, :, :, 16:32], func=CP)
                        oh = pool.tile([128, W, 2, GB], bf, tag="oh", name="oh",
                                       bufs=6)
                        nc.vector.tensor_tensor(
                            oh[:],
                            tct8[:, 2 * pp:2 * pp + 2, :].unsqueeze(1)
                                .to_broadcast([128, W, 2, GB]),
                            it[:],
                            op=EQ,
                        )
                        msg = pool.tile([128, 2, GB, 16], bf, tag="msg",
                                        name="msg", bufs=6)
                        nc.gpsimd.tensor_tensor(msg[:], sg[:], vb[:], op=MUL)
                        ohs.append(oh)
                        msgs.append(msg)
                    # scatter + fused skip matmuls: one accumulation group
                    # per window, consecutive groups alternate PSUM halves
                    for pp in range(SC // 2):
                        for i in range(2):
                            gg = ss * SC + 2 * pp + i
                            for wi in range(2):
                                win_b = gg * 2 + wi
                                pos = 64 * wi
                                col = win_b // 2
                                out_ap = agg[pos:pos + 64, col, :]
                                for b in range(B):
                                    blk = wi * B + b
                                    nc.tensor.matmul(out_ap,
                                                     lhsT=ohs[pp][:, :, i, blk],
                                                     rhs=msgs[pp][:, i, blk, :],
                                                     start=(b == 0), stop=False,
                                                     tile_position=(0, pos))
                                nc.tensor.matmul(out_ap, lhsT=xsk[:, win_b, :],
                                                 rhs=wsb[:, 0:16],
                                                 start=False, stop=True,
                                                 tile_position=(0, pos))
                hsb = hpool.tile([128, 32, 16], f32, tag="hsb")
                nc.scalar.activation(hsb[:], agg[:], func=CP)
                hsbs.append(hsb)
                hsq = pool.tile([128, 32, 16], bf, tag="hsq")
                nc.scalar.activation(hsq[:], agg[:], func=SQ)
                nc.tensor.matmul(sstat[0:1, :], lhsT=ones[:], rhs=hsb[:],
                                 start=(k == 0), stop=(k == NBANK - 1),
                                 tile_position=(0, 0), skip_group_check=True)
                nc.tensor.matmul(sstat[32:33, :], lhsT=onesb[:], rhs=hsq[:],
                                 start=(k == 0), stop=(k == NBANK - 1),
                                 tile_position=(0, 32), skip_group_check=True)

            # ---- stats AllReduce across the 8 cores ----
            stsb0 = pool.tile([1, 512], f32, tag="stsb0")
            nc.vector.tensor_copy(stsb0[:], sstat[0:1, :])
            nc.sync.dma_start(CCIN[0:1, 0:512], stsb0[:])
            stsb1 = pool.tile([1, 512], f32, tag="stsb1")
            nc.vector.tensor_copy(stsb1[:], sstat[32:33, :])
            nc.sync.dma_start(CCIN[0:1, 512:1024], stsb1[:])
            nc.gpsimd.collective_compute(
                "AllReduce", ADD, replica_groups=[list(range(NC))],
                ins=[CCIN[:]], outs=[CCOUT[:]],
            )
            # sred layout: [b(2) x col(32) x f(16)], b=0 sum(h), b=1 sum(h^2)
            sred = pool.tile([1, 2, 32, 16], f32, tag="sred")
            nc.sync.dma_start(sred[:], CCOUT[:])
            nc.sync.dma_start(STATS[:], sred[:])

            # ---- on-device scale/shift: binary-tree col reduction ----
            tr1 = pool.tile([1, 2, 16, 16], f32, tag="tr1")
            nc.vector.tensor_tensor(tr1[:], sred[:, :, 0:16, :],
                                    sred[:, :, 16:32, :], op=ADD)
            tr2 = pool.tile([1, 2, 8, 16], f32, tag="tr2")
            nc.vector.tensor_tensor(tr2[:], tr1[:, :, 0:8, :],
                                    tr1[:, :, 8:16, :], op=ADD)
            tr3 = pool.tile([1, 2, 4, 16], f32, tag="tr3")
            nc.vector.tensor_tensor(tr3[:], tr2[:, :, 0:4, :],
                                    tr2[:, :, 4:8, :], op=ADD)
            tr4 = pool.tile([1, 2, 2, 16], f32, tag="tr4")
            nc.vector.tensor_tensor(tr4[:], tr3[:, :, 0:2, :],
                                    tr3[:, :, 2:4, :], op=ADD)
            ms = pool.tile([1, 2, 16], f32, tag="ms")   # [mean | E[h^2]]
            nc.vector.tensor_tensor(ms[:].unsqueeze(2),
                                    tr4[:, :, 0:1, :], tr4[:, :, 1:2, :], op=ADD)
            nc.vector.tensor_scalar(ms[:], ms[:], 1.0 / N, None, op0=MUL)
            m2 = pool.tile([1, 16], f32, tag="m2")
            nc.vector.tensor_tensor(m2[:], ms[:, 0, :], ms[:, 0, :], op=MUL)
            ve = pool.tile([1, 16], f32, tag="ve")
            nc.vector.tensor_tensor(ve[:], ms[:, 1, :], m2[:], op=SUB)
            nc.vector.tensor_scalar(ve[:], ve[:], BN_EPS, None, op0=ADD)
            inv = pool.tile([1, 16], f32, tag="inv")
            nc.vector.reciprocal(inv[:], ve[:])
            rs = pool.tile([1, 16], f32, tag="rs")
            nc.scalar.activation(rs[:], inv[:], func=SQRT)
            ss2 = pool.tile([1, 32], f32, tag="ss2")    # [scale | shift]
            nc.vector.tensor_tensor(ss2[:, 0:16], rs[:], wsb[0:1, 16:32], op=MUL)
            msc = pool.tile([1, 16], f32, tag="msc")
            nc.vector.tensor_tensor(msc[:], ms[:, 0, :], ss2[:, 0:16], op=MUL)
            nc.vector.tensor_tensor(ss2[:, 16:32], wsb[0:1, 32:48], msc[:], op=SUB)
            bcp = pm.tile([128, 32], f32, space="PSUM", tag="mm", name="bcp")
            nc.tensor.matmul(bcp[:], lhsT=ones1[:], rhs=ss2[:],
                             start=True, stop=True, tile_position=(0, 0),
                             skip_group_check=True)
            bcs = cpool.tile([128, 32], f32)
            nc.scalar.activation(bcs[:], bcp[:], func=CP)
            scl_b = bcs[:, 0:16].unsqueeze(1).to_broadcast([128, 32, 16])
            sft_b = bcs[:, 16:32].unsqueeze(1).to_broadcast([128, 32, 16])

            # ---- BN apply + ReLU + residual from SBUF-resident h ----
            for k in range(NBANK):
                hsb = hsbs[k]
                nc.vector.tensor_tensor(hsb[:], hsb[:], scl_b, op=MUL)
                nc.vector.tensor_tensor(hsb[:], hsb[:], sft_b, op=ADD)
                nc.scalar.activation(hsb[:], hsb[:], func=RELU)
                yb = tpool.tile([128, 32, 16], f32, tag="yb")
                nc.gpsimd.tensor_tensor(yb[:], hsb[:], xbs[k][:], op=ADD)
                nc.sync.dma_start(Y[k], yb[:])
    nc.compile()
    return nc


def host_prep(x, edge_index, edge_attr):
    """Degree-balanced window packing + per-core device array layout."""
    global _gslot
    src = np.asarray(edge_index[0], dtype=np.int64)
    tgt = np.asarray(edge_index[1], dtype=np.int64)
    x = np.asarray(x, dtype=np.float32)
    ea = np.asarray(edge_attr, dtype=np.float32)

    # --- node -> (window, pos) by round-robin deal of degree-sorted nodes ---
    deg = np.bincount(tgt, minlength=N)
    order = np.argsort(-deg, kind="stable")
    node2win = np.empty(N, np.int64)
    node2pos = np.empty(N, np.int64)
    node2win[order] = np.arange(N, dtype=np.int64) % NWIN
    node2pos[order] = np.arange(N, dtype=np.int64) // NWIN
    _gslot = node2win * W + node2pos

    wcnt = np.zeros(NWIN, np.int64)
    np.add.at(wcnt, node2win, deg)
    B = max(1, int(math.ceil(wcnt.max() / 128)))
    S_w = 128 * B
    S = NWIN * S_w
    S_c = PW * S_w

    # --- edge slots: sort edges by target window ---
    ew = node2win[tgt]
    perm = np.argsort(ew, kind="stable")
    ew_s = ew[perm]
    starts = np.zeros(NWIN + 1, np.int64)
    starts[1:] = np.cumsum(wcnt)
    slots = ew_s * S_w + (np.arange(E, dtype=np.int64) - starts[ew_s])

    GB = 2 * B
    SC = 8
    nsc_pb = 32 // SC
    nsc = NBANK * nsc_pb

    tgt_s = tgt[perm]
    src_s = src[perm]
    x16 = x.astype(BF16)
    pay = np.zeros((S, 48), BF16)
    pay[slots, 0:16] = x16[tgt_s]
    pay[slots, 16:32] = x16[src_s]
    pay[slots, 32:48] = ea[perm].astype(BF16)

    trel = np.full(S, -1.0, np.float32)
    trel[slots] = node2pos[tgt_s].astype(np.float32)
    trel16 = trel.astype(BF16)

    # x in padded-slot order
    xpad = np.zeros((NPAD, D), np.float32)
    xpad[_gslot] = x
    mask = np.zeros(NPAD, np.float32)
    mask[_gslot] = 1.0

    iota = np.broadcast_to(
        np.arange(W, dtype=np.float32).astype(BF16).reshape(1, W, 1, 1),
        (128, W, 2, GB)).copy()

    xt = x_tiled(xpad)

    in_maps = []
    for c in range(NC):
        inp_c = np.empty((49, S_c), BF16)
        inp_c[0:48] = pay[c * S_c:(c + 1) * S_c].T
        inp_c[48] = BF16(1.0)
        inp_c = inp_c.reshape(49, S_c // 128, 128)
        trel_c = (trel16[c * S_c:(c + 1) * S_c]
                  .reshape(nsc, SC, GB, 128).transpose(0, 3, 1, 2).copy())
        xsk_c = np.empty((17, NODES_C), np.float32)
        xsk_c[0:16] = xpad[c * NODES_C:(c + 1) * NODES_C].T
        xsk_c[16] = mask[c * NODES_C:(c + 1) * NODES_C]
        in_maps.append({
            "inp": inp_c, "trel": trel_c, "iota": iota,
            "xsk": xsk_c, "xt": xt[c],
        })
    return B, in_maps, xpad


def weight_arrays(Wk, bk, Wq, bq, Wv, bv, We, Ws, bs, bias,
                  gamma=None, beta=None):
    wgt = np.zeros((49, 32), np.float32)
    wgt[0:16, 0:16] = Wk
    wgt[16:32, 0:16] = Wq
    wgt[32:48, 0:16] = We
    wgt[48, 0:16] = bk + bq
    wgt[16:32, 16:32] = Wv
    wgt[48, 16:32] = bv
    wsb = np.zeros((17, 48), np.float32)
    wsb[0:16, 0:16] = Ws
    wsb[16, 0:16] = bs + bias
    wsb[0, 16:32] = 1.0 if gamma is None else np.asarray(gamma, np.float32)
    wsb[0, 32:48] = 0.0 if beta is None else np.asarray(beta, np.float32)
    return wgt.astype(BF16), wsb


def x_tiled(xpad):
    # [NC, NBANK, 128, 32, 16]; slot s in core c: s = k*4096 + col*128 + p
    xt = xpad.reshape(NC, NBANK, 32, 128, D).transpose(0, 1, 3, 2, 4)
    return np.ascontiguousarray(xt).astype(BF16)


def untile_y(y_t):
    # y_t: [NC, NBANK, 128, 32, 16] -> [NPAD, 16]
    return y_t.transpose(0, 1, 3, 2, 4).reshape(NPAD, D)


def kernel(**inputs):
    x = np.asarray(inputs["x"], np.float32)
    B, in_maps, xpad = host_prep(x, inputs["edge_index"], inputs["edge_attr"])
    wgt, wsb = weight_arrays(
        np.asarray(inputs["Wk"], np.float32), np.asarray(inputs["bk"], np.float32),
        np.asarray(inputs["Wq"], np.float32), np.asarray(inputs["bq"], np.float32),
        np.asarray(inputs["Wv"], np.float32), np.asarray(inputs["bv"], np.float32),
        np.asarray(inputs["We"], np.float32), np.asarray(inputs["Ws"], np.float32),
        np.asarray(inputs["bs"], np.float32), np.asarray(inputs["bias"], np.float32),
        np.asarray(inputs["gamma"], np.float32), np.asarray(inputs["beta"], np.float32))
    for m in in_maps:
        m["wgt"] = wgt
        m["wsb"] = wsb

    if ("p1", B) not in _nc_cache:
        _nc_cache[("p1", B)] = _build_phase1(B)
    nc1 = _nc_cache[("p1", B)]
    res1 = run_bass_kernel_spmd(nc1, in_maps, list(range(NC)))

    y_t = np.stack([res1.results[c]["y"] for c in range(NC)])
    y = untile_y(y_t)[_gslot]
    return y.astype(np.float32)
